# revision 31
# baseline (speedup 1.0000x reference)
"""Trainium2 Bass kernel for nn_Linear_8589934906 (gnn_message_passing).

y[n, f] = sum_j w_table[widx[n], j] * pool[idx[n, j], f]
  N=500_000 neurons, P=16 inputs/neuron, F=32 features,
  pool = concat(values0, values1) = [400_000, 32] f32, w_table = [10_000, 16].

The metric is the warm wall-clock of kernel(); the axon H2D/D2H tunnel runs
at only ~35-70 MB/s, so the design minimizes host<->device bytes first:
  - pool + w_table cast to bf16 on host (tolerance gate is 2e-2; bf16 adds
    ~5e-3), pool uploaded as one [50_000, 32] shard per core and replicated
    on-device with an AllGather (upload 26 MB instead of 410 MB).
  - idx (19-bit values) ships packed as u16 lo + u8 hi and is reconstructed
    on-device by DVE (exact: hi*65536+lo < 2^24); widx ships as u16.
  - output returned as bf16 [N, 32], cast to f32 on host.

Device program per core, data-parallel over N (8 cores x 62_500 neurons):
  - Prologue: shard -> DRAM bounce -> AllGather -> full bf16 pool in DRAM.
  - Per tile (128 partitions x C=16 neurons/partition = 2048 neurons):
      * HWDGE load idxlo/idxhi/widx tiles; DVE rebuilds i32 offsets
      * SWDGE indirect gathers: HW supports exactly one descriptor per
        partition per instruction (offset AP [128,1], dest [128, F]
        contiguous; anything fancier is ignored or crashes the exec unit),
        so C*P=256 gather instructions round-robined over 4 SWDGE queues
      * DVE: G *= broadcast(W); tensor_reduce over j -> bf16 y tile
      * HWDGE store y tile
  - Fully unrolled (no For_i: the loop back-edge drain serializes the DMA
    pipeline, measured +0.9 s device time for -0.25 s host lowering).
"""

import os
import sys

import numpy as np

if "/opt/trn_rl_repo" not in sys.path:
    sys.path.insert(0, "/opt/trn_rl_repo")

# ---- problem constants (hardcoded; kernel.py must be self-contained) ----
N = 500_000
P = 16
F = 32
M = 200_000
K = 10_000
N_CORES = 8
C = 16                      # neurons per partition per tile
TILE_N = 128 * C            # neurons per tile
N_PER_CORE = (N + N_CORES - 1) // N_CORES          # 62500
T = (N_PER_CORE + TILE_N - 1) // TILE_N            # tiles per core
N_PAD = T * TILE_N                                 # padded neurons per core
GQ = 4                      # indirect-DMA queue splits for the pool gather
BUFS = 3
USE_FOR_I = False           # hardware loop shrinks the BIR ~23x but the
                            # back-edge drain serializes the DMA pipeline:
                            # measured +0.9s device time vs -0.25s host. Off.

# set by test.py to capture an NTFF profile on the next kernel() call
TRACE = False
LAST_RESULTS = None


def build_program(t_tiles, c, pool_rows, wtab_rows, bufs=BUFS, gq=GQ):
    """Build the SPMD Bass program for one core: t_tiles tiles of 128*c neurons.

    The pool is uploaded as one [pool_rows/8, F] shard per core and
    replicated on-device via AllGather (the axon H2D tunnel is ~70 MB/s,
    so shipping 8 replicas from the host dominated the wall time).
    """
    import concourse.bacc as bacc
    import concourse.bass as bass
    import concourse.mybir as mybir
    from concourse.tile import TileContext

    f32 = mybir.dt.float32
    bf16 = mybir.dt.bfloat16
    i32 = mybir.dt.int32
    u16 = mybir.dt.uint16
    u8 = mybir.dt.uint8
    rows = t_tiles * 128
    cp = c * P
    del gq  # descriptor-per-partition HW limit makes queue splits moot
    shard_rows = pool_rows // N_CORES

    nc = bacc.Bacc("TRN2", target_bir_lowering=False, debug=True,
                   num_swdge_queues=4, num_devices=N_CORES,
                   disable_frame_to_traceback=True)
    poolsh_d = nc.dram_tensor("poolsh", [shard_rows, F], bf16,
                              kind="ExternalInput")
    wtabsh_d = nc.dram_tensor("wtabsh", [wtab_rows // N_CORES, P], bf16,
                              kind="ExternalInput")
    # all integer inputs ride in ONE u8 array (each separate H2D array pays
    # ~15-60 ms of tunnel latency): per row, bytes [0:2cp) = idx lo u16,
    # [2cp:3cp) = idx hi u8, [3cp:3cp+2c) = widx u16. idx = hi*65536 + lo
    # is rebuilt on-device (exact: 400000 < 2^24).
    pk_lo, pk_hi, pk_w = 0, 2 * cp, 3 * cp
    pk_end = pk_w + 2 * c
    packed_d = nc.dram_tensor("packed", [rows, pk_end], u8,
                              kind="ExternalInput")
    y_d = nc.dram_tensor("y", [rows, c * F], bf16, kind="ExternalOutput")
    del u16

    with TileContext(nc) as tc:
        with tc.tile_pool(name="dram", bufs=1, space="DRAM") as dram, \
             tc.tile_pool(name="gbuf", bufs=bufs) as gpool, \
             tc.tile_pool(name="wbuf", bufs=bufs) as wpool, \
             tc.tile_pool(name="ibuf", bufs=bufs) as ipool, \
             tc.tile_pool(name="ybuf", bufs=bufs) as ypool:
            # replicate pool + w_table on-device: shard -> bounce -> AllGather
            cc_in = dram.tile([shard_rows, F], bf16)
            pool_d = dram.tile([pool_rows, F], bf16, addr_space="Shared")
            nc.gpsimd.dma_start(cc_in[:], poolsh_d[:])
            nc.gpsimd.collective_compute(
                "AllGather", mybir.AluOpType.bypass,
                replica_groups=[list(range(N_CORES))],
                ins=[cc_in.opt()], outs=[pool_d.opt()],
            )
            ccw_in = dram.tile([wtab_rows // N_CORES, P], bf16)
            wtab_d = dram.tile([wtab_rows, P], bf16, addr_space="Shared")
            nc.gpsimd.dma_start(ccw_in[:], wtabsh_d[:])
            nc.gpsimd.collective_compute(
                "AllGather", mybir.AluOpType.bypass,
                replica_groups=[list(range(N_CORES))],
                ins=[ccw_in.opt()], outs=[wtab_d.opt()],
            )
            def tile_body(r0):
                rs = bass.ds(r0, 128)
                raw = ipool.tile([128, pk_end], u8, tag="raw")
                nc.sync.dma_start(out=raw[:], in_=packed_d[rs, :])
                r3 = raw[:]

                # rebuild i32 offsets from little-endian bytes:
                # it = lo_even + 256*lo_odd, then += 65536*hi (exact in fp32)
                it = ipool.tile([128, cp], i32, tag="it")
                nc.vector.scalar_tensor_tensor(
                    out=it[:], in0=r3[:, pk_lo + 1:pk_hi:2], scalar=256,
                    in1=r3[:, pk_lo:pk_hi:2],
                    op0=mybir.AluOpType.mult, op1=mybir.AluOpType.add)
                nc.vector.scalar_tensor_tensor(
                    out=it[:], in0=r3[:, pk_hi:pk_w], scalar=65536,
                    in1=it[:],
                    op0=mybir.AluOpType.mult, op1=mybir.AluOpType.add)
                wit = ipool.tile([128, c], i32, tag="wit")
                nc.vector.scalar_tensor_tensor(
                    out=wit[:], in0=r3[:, pk_w + 1:pk_end:2], scalar=256,
                    in1=r3[:, pk_w:pk_end:2],
                    op0=mybir.AluOpType.mult, op1=mybir.AluOpType.add)

                # pool gather: HW allows one descriptor per partition per
                # indirect DMA (offset AP [128,1], dest [128, F] contiguous),
                # so issue c*P instructions round-robined over 4 SWDGE queues
                g = gpool.tile([128, cp * F], bf16, tag="g")
                for s in range(cp):
                    inst = nc.gpsimd.indirect_dma_start(
                        out=g[:, s * F:(s + 1) * F], out_offset=None,
                        in_=pool_d[:],
                        in_offset=bass.IndirectOffsetOnAxis(
                            ap=it[:, s:s + 1], axis=0),
                    )
                    qi = s % 4
                    if qi:
                        inst.queue = f"qPoolDynamic{qi}"

                # w gather: c instructions of 128 descriptors x P*2 bytes
                w = wpool.tile([128, cp], bf16, tag="w")
                for s in range(c):
                    nc.gpsimd.indirect_dma_start(
                        out=w[:, s * P:(s + 1) * P], out_offset=None,
                        in_=wtab_d[:],
                        in_offset=bass.IndirectOffsetOnAxis(
                            ap=wit[:, s:s + 1], axis=0),
                    )

                # weighted multiply: g[p, sj, f] *= w[p, sj] (broadcast over f)
                g3 = g[:].rearrange("p (sj f) -> p sj f", sj=cp, f=F)
                w3 = w[:].unsqueeze(2).to_broadcast([128, cp, F])
                nc.vector.tensor_tensor(
                    out=g3, in0=g3, in1=w3, op=mybir.AluOpType.mult)

                # reduce over j (strided innermost view): [p, s, f, j] -> [p, s*f]
                # bf16 out is a final rounding only (DVE accumulates in f32);
                # harness tolerance is 2e-2, bf16 costs ~4e-3
                y_t = ypool.tile([128, c * F], bf16, tag="y")
                g4 = g[:].rearrange("p (s j f) -> p s f j", s=c, j=P, f=F)
                with nc.allow_low_precision(reason="bf16 output, 2e-2 gate"):
                    nc.vector.tensor_reduce(
                        out=y_t[:], in_=g4,
                        axis=mybir.AxisListType.X, op=mybir.AluOpType.add)

                nc.sync.dma_start(out=y_d[rs, :], in_=y_t[:])

            if USE_FOR_I:
                with tc.For_i(0, rows, 128,
                              hint_engines=(mybir.EngineType.Pool,)) as i:
                    tile_body(i)
            else:
                for t in range(t_tiles):
                    tile_body(t * 128)
    nc.finalize()
    return nc


def _prep_core_inputs(idxlo, idxhi, widx16, n0, n1, t_tiles, c):
    """Slice per-core indices, pad, reshape, and pack into one u8 array:
    per row, bytes [0:2cp) = idx lo u16, [2cp:3cp) = idx hi, [3cp:) = widx."""
    npad = t_tiles * 128 * c
    rows = t_tiles * 128
    cp = c * P
    lo_c = np.zeros((npad, P), np.uint16)
    lo_c[: n1 - n0] = idxlo[n0:n1]
    hi_c = np.zeros((npad, P), np.uint8)
    hi_c[: n1 - n0] = idxhi[n0:n1]
    w_c = np.zeros((npad,), np.uint16)
    w_c[: n1 - n0] = widx16[n0:n1]
    # neuron m = (t*128 + p)*c + s  ->  idx tile [t*128+p, s*16+j]
    packed = np.empty((rows, 3 * cp + 2 * c), np.uint8)
    packed[:, :2 * cp] = lo_c.reshape(rows, cp).view(np.uint8)
    packed[:, 2 * cp:3 * cp] = hi_c.reshape(rows, cp)
    packed[:, 3 * cp:] = w_c.reshape(rows, c).view(np.uint8)
    return packed


_NC_CACHE = {}


def _enable_jax_compile_cache():
    """Persistent XLA compilation cache so warm calls skip recompiling the
    shard_map wrapper that run_bass_via_pjrt rebuilds per call."""
    try:
        import jax

        jax.config.update("jax_compilation_cache_dir", "/tmp/jaxcache")
        jax.config.update("jax_persistent_cache_min_entry_size_bytes", -1)
        jax.config.update("jax_persistent_cache_min_compile_time_secs", 0.0)
    except Exception:
        pass


_enable_jax_compile_cache()


def kernel(values0, values1, w_table, idx, widx):
    global LAST_RESULTS
    import time as _time

    timing = bool(os.environ.get("KERNEL_TIMING"))
    tick = _time.time
    t0 = tick()
    from concourse.bass_utils import run_bass_kernel_spmd

    import ml_dtypes

    bf16 = np.dtype(ml_dtypes.bfloat16)
    pool = np.concatenate([np.asarray(values0, np.float32).astype(bf16),
                           np.asarray(values1, np.float32).astype(bf16)],
                          axis=0)
    w_table = np.asarray(w_table, np.float32).astype(bf16)
    ta = tick()
    idx32 = np.asarray(idx).astype(np.int32)
    idxlo = (idx32 & 0xFFFF).astype(np.uint16)
    idxhi = (idx32 >> 16).astype(np.uint8)
    widx16 = np.asarray(widx).astype(np.uint16)
    t1 = tick()

    if "nc" not in _NC_CACHE:
        _NC_CACHE["nc"] = build_program(T, C, 2 * M, K)
    nc = _NC_CACHE["nc"]
    t2 = tick()

    shard_rows = (2 * M) // N_CORES
    wsh_rows = K // N_CORES
    in_maps = []
    for core in range(N_CORES):
        n0 = core * N_PER_CORE
        n1 = min(n0 + N_PER_CORE, N)
        packed = _prep_core_inputs(idxlo, idxhi, widx16, n0, n1, T, C)
        in_maps.append({"poolsh": pool[core * shard_rows:(core + 1) * shard_rows],
                        "wtabsh": w_table[core * wsh_rows:(core + 1) * wsh_rows],
                        "packed": packed})
    t3 = tick()

    kwargs = {}
    if TRACE:
        kwargs = {"trace": True, "trace_cores": [0]}
    res = run_bass_kernel_spmd(nc, in_maps, core_ids=list(range(N_CORES)),
                               **kwargs)
    LAST_RESULTS = res
    t4 = tick()

    out = np.empty((N, F), np.float32)
    for core in range(N_CORES):
        n0 = core * N_PER_CORE
        n1 = min(n0 + N_PER_CORE, N)
        y_t = res.results[core]["y"].reshape(N_PAD, F)
        out[n0:n1] = y_t[: n1 - n0]  # numpy casts bf16->f32 on assignment
    t5 = tick()
    if timing:
        print(f"[kernel timing] pool/w cast={ta-t0:.3f}s idx pack={t1-ta:.3f}s "
              f"build={t2-t1:.3f}s prep={t3-t2:.3f}s run_spmd={t4-t3:.3f}s "
              f"unshard={t5-t4:.3f}s", flush=True)
    return out


if __name__ == "__main__":
    # quick shape sanity
    print(f"T={T} tiles/core, C={C}, N_PAD={N_PAD} vs N_PER_CORE={N_PER_CORE}")


# revision 32
# speedup vs baseline: 1.0152x; 1.0152x over previous
"""Trainium2 Bass kernel for nn_Linear_8589934906 (gnn_message_passing).

y[n, f] = sum_j w_table[widx[n], j] * pool[idx[n, j], f]
  N=500_000 neurons, P=16 inputs/neuron, F=32 features,
  pool = concat(values0, values1) = [400_000, 32] f32, w_table = [10_000, 16].

The metric is the warm wall-clock of kernel(); the axon H2D/D2H tunnel runs
at only ~35-70 MB/s, so the design minimizes host<->device bytes first:
  - pool + w_table cast to bf16 on host (tolerance gate is 2e-2; bf16 adds
    ~5e-3), pool uploaded as one [50_000, 32] shard per core and replicated
    on-device with an AllGather (upload 26 MB instead of 410 MB).
  - idx (19-bit values) ships packed as u16 lo + u8 hi and is reconstructed
    on-device by DVE (exact: hi*65536+lo < 2^24); widx ships as u16.
  - output returned as bf16 [N, 32], cast to f32 on host.

Device program per core, data-parallel over N (8 cores x 62_500 neurons):
  - Prologue: shard -> DRAM bounce -> AllGather -> full bf16 pool in DRAM.
  - Per tile (128 partitions x C=16 neurons/partition = 2048 neurons):
      * HWDGE load idxlo/idxhi/widx tiles; DVE rebuilds i32 offsets
      * SWDGE indirect gathers: HW supports exactly one descriptor per
        partition per instruction (offset AP [128,1], dest [128, F]
        contiguous; anything fancier is ignored or crashes the exec unit),
        so C*P=256 gather instructions round-robined over 4 SWDGE queues
      * DVE: G *= broadcast(W); tensor_reduce over j -> bf16 y tile
      * HWDGE store y tile
  - Fully unrolled (no For_i: the loop back-edge drain serializes the DMA
    pipeline, measured +0.9 s device time for -0.25 s host lowering).
"""

import os
import sys

import numpy as np

if "/opt/trn_rl_repo" not in sys.path:
    sys.path.insert(0, "/opt/trn_rl_repo")

# ---- problem constants (hardcoded; kernel.py must be self-contained) ----
N = 500_000
P = 16
F = 32
M = 200_000
K = 10_000
N_CORES = 8
C = 16                      # neurons per partition per tile
TILE_N = 128 * C            # neurons per tile
N_PER_CORE = (N + N_CORES - 1) // N_CORES          # 62500
T = (N_PER_CORE + TILE_N - 1) // TILE_N            # tiles per core
N_PAD = T * TILE_N                                 # padded neurons per core
GQ = 4                      # indirect-DMA queue splits for the pool gather
BUFS = 3
USE_FOR_I = False           # hardware loop shrinks the BIR ~23x but the
                            # back-edge drain serializes the DMA pipeline:
                            # measured +0.9s device time vs -0.25s host. Off.

# set by test.py to capture an NTFF profile on the next kernel() call
TRACE = False
LAST_RESULTS = None


def build_program(t_tiles, c, pool_rows, wtab_rows, bufs=BUFS, gq=GQ):
    """Build the SPMD Bass program for one core: t_tiles tiles of 128*c neurons.

    The pool is uploaded as one [pool_rows/8, F] shard per core and
    replicated on-device via AllGather (the axon H2D tunnel is ~70 MB/s,
    so shipping 8 replicas from the host dominated the wall time).
    """
    import concourse.bacc as bacc
    import concourse.bass as bass
    import concourse.mybir as mybir
    from concourse.tile import TileContext

    f32 = mybir.dt.float32
    bf16 = mybir.dt.bfloat16
    i32 = mybir.dt.int32
    u16 = mybir.dt.uint16
    u8 = mybir.dt.uint8
    rows = t_tiles * 128
    cp = c * P
    del gq  # descriptor-per-partition HW limit makes queue splits moot
    shard_rows = pool_rows // N_CORES

    nc = bacc.Bacc("TRN2", target_bir_lowering=False, debug=False,
                   num_swdge_queues=4, num_devices=N_CORES,
                   disable_frame_to_traceback=True)
    poolsh_d = nc.dram_tensor("poolsh", [shard_rows, F], bf16,
                              kind="ExternalInput")
    wtabsh_d = nc.dram_tensor("wtabsh", [wtab_rows // N_CORES, P], bf16,
                              kind="ExternalInput")
    # all integer inputs ride in ONE u8 array (each separate H2D array pays
    # ~15-60 ms of tunnel latency): per row, bytes [0:2cp) = idx lo u16,
    # [2cp:3cp) = idx hi u8, [3cp:3cp+2c) = widx u16. idx = hi*65536 + lo
    # is rebuilt on-device (exact: 400000 < 2^24).
    pk_lo, pk_hi, pk_w = 0, 2 * cp, 3 * cp
    pk_end = pk_w + 2 * c
    packed_d = nc.dram_tensor("packed", [rows, pk_end], u8,
                              kind="ExternalInput")
    y_d = nc.dram_tensor("y", [rows, c * F], bf16, kind="ExternalOutput")
    del u16

    with TileContext(nc) as tc:
        with tc.tile_pool(name="dram", bufs=1, space="DRAM") as dram, \
             tc.tile_pool(name="gbuf", bufs=bufs) as gpool, \
             tc.tile_pool(name="wbuf", bufs=bufs) as wpool, \
             tc.tile_pool(name="ibuf", bufs=bufs) as ipool, \
             tc.tile_pool(name="ybuf", bufs=bufs) as ypool:
            # replicate pool + w_table on-device: shard -> bounce -> AllGather
            cc_in = dram.tile([shard_rows, F], bf16)
            pool_d = dram.tile([pool_rows, F], bf16, addr_space="Shared")
            nc.gpsimd.dma_start(cc_in[:], poolsh_d[:])
            nc.gpsimd.collective_compute(
                "AllGather", mybir.AluOpType.bypass,
                replica_groups=[list(range(N_CORES))],
                ins=[cc_in.opt()], outs=[pool_d.opt()],
            )
            ccw_in = dram.tile([wtab_rows // N_CORES, P], bf16)
            wtab_d = dram.tile([wtab_rows, P], bf16, addr_space="Shared")
            nc.gpsimd.dma_start(ccw_in[:], wtabsh_d[:])
            nc.gpsimd.collective_compute(
                "AllGather", mybir.AluOpType.bypass,
                replica_groups=[list(range(N_CORES))],
                ins=[ccw_in.opt()], outs=[wtab_d.opt()],
            )
            def tile_body(r0):
                rs = bass.ds(r0, 128)
                raw = ipool.tile([128, pk_end], u8, tag="raw")
                nc.sync.dma_start(out=raw[:], in_=packed_d[rs, :])
                r3 = raw[:]

                # rebuild i32 offsets from little-endian bytes:
                # it = lo_even + 256*lo_odd, then += 65536*hi (exact in fp32)
                it = ipool.tile([128, cp], i32, tag="it")
                nc.vector.scalar_tensor_tensor(
                    out=it[:], in0=r3[:, pk_lo + 1:pk_hi:2], scalar=256,
                    in1=r3[:, pk_lo:pk_hi:2],
                    op0=mybir.AluOpType.mult, op1=mybir.AluOpType.add)
                nc.vector.scalar_tensor_tensor(
                    out=it[:], in0=r3[:, pk_hi:pk_w], scalar=65536,
                    in1=it[:],
                    op0=mybir.AluOpType.mult, op1=mybir.AluOpType.add)
                wit = ipool.tile([128, c], i32, tag="wit")
                nc.vector.scalar_tensor_tensor(
                    out=wit[:], in0=r3[:, pk_w + 1:pk_end:2], scalar=256,
                    in1=r3[:, pk_w:pk_end:2],
                    op0=mybir.AluOpType.mult, op1=mybir.AluOpType.add)

                # pool gather: HW allows one descriptor per partition per
                # indirect DMA (offset AP [128,1], dest [128, F] contiguous),
                # so issue c*P instructions round-robined over 4 SWDGE queues
                g = gpool.tile([128, cp * F], bf16, tag="g")
                for s in range(cp):
                    inst = nc.gpsimd.indirect_dma_start(
                        out=g[:, s * F:(s + 1) * F], out_offset=None,
                        in_=pool_d[:],
                        in_offset=bass.IndirectOffsetOnAxis(
                            ap=it[:, s:s + 1], axis=0),
                    )
                    qi = s % 4
                    if qi:
                        inst.queue = f"qPoolDynamic{qi}"

                # w gather: c instructions of 128 descriptors x P*2 bytes
                w = wpool.tile([128, cp], bf16, tag="w")
                for s in range(c):
                    nc.gpsimd.indirect_dma_start(
                        out=w[:, s * P:(s + 1) * P], out_offset=None,
                        in_=wtab_d[:],
                        in_offset=bass.IndirectOffsetOnAxis(
                            ap=wit[:, s:s + 1], axis=0),
                    )

                # weighted multiply: g[p, sj, f] *= w[p, sj] (broadcast over f)
                g3 = g[:].rearrange("p (sj f) -> p sj f", sj=cp, f=F)
                w3 = w[:].unsqueeze(2).to_broadcast([128, cp, F])
                nc.vector.tensor_tensor(
                    out=g3, in0=g3, in1=w3, op=mybir.AluOpType.mult)

                # reduce over j (strided innermost view): [p, s, f, j] -> [p, s*f]
                # bf16 out is a final rounding only (DVE accumulates in f32);
                # harness tolerance is 2e-2, bf16 costs ~4e-3
                y_t = ypool.tile([128, c * F], bf16, tag="y")
                g4 = g[:].rearrange("p (s j f) -> p s f j", s=c, j=P, f=F)
                with nc.allow_low_precision(reason="bf16 output, 2e-2 gate"):
                    nc.vector.tensor_reduce(
                        out=y_t[:], in_=g4,
                        axis=mybir.AxisListType.X, op=mybir.AluOpType.add)

                nc.sync.dma_start(out=y_d[rs, :], in_=y_t[:])

            if USE_FOR_I:
                with tc.For_i(0, rows, 128,
                              hint_engines=(mybir.EngineType.Pool,)) as i:
                    tile_body(i)
            else:
                for t in range(t_tiles):
                    tile_body(t * 128)
    nc.finalize()
    return nc


def _prep_core_inputs(idxlo, idxhi, widx16, n0, n1, t_tiles, c):
    """Slice per-core indices, pad, reshape, and pack into one u8 array:
    per row, bytes [0:2cp) = idx lo u16, [2cp:3cp) = idx hi, [3cp:) = widx."""
    npad = t_tiles * 128 * c
    rows = t_tiles * 128
    cp = c * P
    lo_c = np.zeros((npad, P), np.uint16)
    lo_c[: n1 - n0] = idxlo[n0:n1]
    hi_c = np.zeros((npad, P), np.uint8)
    hi_c[: n1 - n0] = idxhi[n0:n1]
    w_c = np.zeros((npad,), np.uint16)
    w_c[: n1 - n0] = widx16[n0:n1]
    # neuron m = (t*128 + p)*c + s  ->  idx tile [t*128+p, s*16+j]
    packed = np.empty((rows, 3 * cp + 2 * c), np.uint8)
    packed[:, :2 * cp] = lo_c.reshape(rows, cp).view(np.uint8)
    packed[:, 2 * cp:3 * cp] = hi_c.reshape(rows, cp)
    packed[:, 3 * cp:] = w_c.reshape(rows, c).view(np.uint8)
    return packed


_NC_CACHE = {}


def _enable_jax_compile_cache():
    """Persistent XLA compilation cache so warm calls skip recompiling the
    shard_map wrapper that run_bass_via_pjrt rebuilds per call."""
    try:
        import jax

        jax.config.update("jax_compilation_cache_dir", "/tmp/jaxcache")
        jax.config.update("jax_persistent_cache_min_entry_size_bytes", -1)
        jax.config.update("jax_persistent_cache_min_compile_time_secs", 0.0)
    except Exception:
        pass


_enable_jax_compile_cache()


def kernel(values0, values1, w_table, idx, widx):
    global LAST_RESULTS
    import time as _time

    timing = bool(os.environ.get("KERNEL_TIMING"))
    tick = _time.time
    t0 = tick()
    from concourse.bass_utils import run_bass_kernel_spmd

    import ml_dtypes

    bf16 = np.dtype(ml_dtypes.bfloat16)
    pool = np.concatenate([np.asarray(values0, np.float32).astype(bf16),
                           np.asarray(values1, np.float32).astype(bf16)],
                          axis=0)
    w_table = np.asarray(w_table, np.float32).astype(bf16)
    ta = tick()
    idx32 = np.asarray(idx).astype(np.int32)
    idxlo = (idx32 & 0xFFFF).astype(np.uint16)
    idxhi = (idx32 >> 16).astype(np.uint8)
    widx16 = np.asarray(widx).astype(np.uint16)
    t1 = tick()

    if "nc" not in _NC_CACHE:
        _NC_CACHE["nc"] = build_program(T, C, 2 * M, K)
    nc = _NC_CACHE["nc"]
    t2 = tick()

    shard_rows = (2 * M) // N_CORES
    wsh_rows = K // N_CORES
    in_maps = []
    for core in range(N_CORES):
        n0 = core * N_PER_CORE
        n1 = min(n0 + N_PER_CORE, N)
        packed = _prep_core_inputs(idxlo, idxhi, widx16, n0, n1, T, C)
        in_maps.append({"poolsh": pool[core * shard_rows:(core + 1) * shard_rows],
                        "wtabsh": w_table[core * wsh_rows:(core + 1) * wsh_rows],
                        "packed": packed})
    t3 = tick()

    kwargs = {}
    if TRACE:
        kwargs = {"trace": True, "trace_cores": [0]}
    res = run_bass_kernel_spmd(nc, in_maps, core_ids=list(range(N_CORES)),
                               **kwargs)
    LAST_RESULTS = res
    t4 = tick()

    out = np.empty((N, F), np.float32)
    for core in range(N_CORES):
        n0 = core * N_PER_CORE
        n1 = min(n0 + N_PER_CORE, N)
        y_t = res.results[core]["y"].reshape(N_PAD, F)
        out[n0:n1] = y_t[: n1 - n0]  # numpy casts bf16->f32 on assignment
    t5 = tick()
    if timing:
        print(f"[kernel timing] pool/w cast={ta-t0:.3f}s idx pack={t1-ta:.3f}s "
              f"build={t2-t1:.3f}s prep={t3-t2:.3f}s run_spmd={t4-t3:.3f}s "
              f"unshard={t5-t4:.3f}s", flush=True)
    return out


if __name__ == "__main__":
    # quick shape sanity
    print(f"T={T} tiles/core, C={C}, N_PAD={N_PAD} vs N_PER_CORE={N_PER_CORE}")


# revision 33
# speedup vs baseline: 1.1512x; 1.1339x over previous
"""Trainium2 Bass kernel for nn_Linear_8589934906 (gnn_message_passing).

y[n, f] = sum_j w_table[widx[n], j] * pool[idx[n, j], f]
  N=500_000 neurons, P=16 inputs/neuron, F=32 features,
  pool = concat(values0, values1) = [400_000, 32] f32, w_table = [10_000, 16].

The metric is the warm wall-clock of kernel(); the axon H2D/D2H tunnel runs
at only ~35-70 MB/s, so the design minimizes host<->device bytes first:
  - pool + w_table cast to bf16 on host (tolerance gate is 2e-2; bf16 adds
    ~5e-3), pool uploaded as one [50_000, 32] shard per core and replicated
    on-device with an AllGather (upload 26 MB instead of 410 MB).
  - idx (19-bit values) ships packed as u16 lo + u8 hi and is reconstructed
    on-device by DVE (exact: hi*65536+lo < 2^24); widx ships as u16.
  - output returned as bf16 [N, 32], cast to f32 on host.

Device program per core, data-parallel over N (8 cores x 62_500 neurons):
  - Prologue: shard -> DRAM bounce -> AllGather -> full bf16 pool in DRAM.
  - Per tile (128 partitions x C=16 neurons/partition = 2048 neurons):
      * HWDGE load idxlo/idxhi/widx tiles; DVE rebuilds i32 offsets
      * SWDGE indirect gathers: HW supports exactly one descriptor per
        partition per instruction (offset AP [128,1], dest [128, F]
        contiguous; anything fancier is ignored or crashes the exec unit),
        so C*P=256 gather instructions round-robined over 4 SWDGE queues
      * DVE: G *= broadcast(W); tensor_reduce over j -> bf16 y tile
      * HWDGE store y tile
  - Fully unrolled (no For_i: the loop back-edge drain serializes the DMA
    pipeline, measured +0.9 s device time for -0.25 s host lowering).
"""

import os
import sys

import numpy as np

if "/opt/trn_rl_repo" not in sys.path:
    sys.path.insert(0, "/opt/trn_rl_repo")

# ---- problem constants (hardcoded; kernel.py must be self-contained) ----
N = 500_000
P = 16
F = 32
M = 200_000
K = 10_000
N_CORES = 8
C = 16                      # neurons per partition per tile
TILE_N = 128 * C            # neurons per tile
N_PER_CORE = (N + N_CORES - 1) // N_CORES          # 62500
T = (N_PER_CORE + TILE_N - 1) // TILE_N            # tiles per core
N_PAD = T * TILE_N                                 # padded neurons per core
GQ = 4                      # indirect-DMA queue splits for the pool gather
BUFS = 3
USE_FOR_I = False           # hardware loop shrinks the BIR ~23x but the
                            # back-edge drain serializes the DMA pipeline:
                            # measured +0.9s device time vs -0.25s host. Off.

# set by test.py to capture an NTFF profile on the next kernel() call
TRACE = False
LAST_RESULTS = None


def build_program(t_tiles, c, pool_rows, wtab_rows, bufs=BUFS, gq=GQ):
    """Build the SPMD Bass program for one core: t_tiles tiles of 128*c neurons.

    The pool is uploaded as one [pool_rows/8, F] shard per core and
    replicated on-device via AllGather (the axon H2D tunnel is ~70 MB/s,
    so shipping 8 replicas from the host dominated the wall time).
    """
    import concourse.bacc as bacc
    import concourse.bass as bass
    import concourse.mybir as mybir
    from concourse.tile import TileContext

    f32 = mybir.dt.float32
    bf16 = mybir.dt.bfloat16
    i32 = mybir.dt.int32
    u16 = mybir.dt.uint16
    u8 = mybir.dt.uint8
    rows = t_tiles * 128
    cp = c * P
    del gq  # descriptor-per-partition HW limit makes queue splits moot
    shard_rows = pool_rows // N_CORES

    nc = bacc.Bacc("TRN2", target_bir_lowering=False, debug=False,
                   num_swdge_queues=4, num_devices=N_CORES,
                   disable_frame_to_traceback=True)
    poolsh_d = nc.dram_tensor("poolsh", [shard_rows, F], bf16,
                              kind="ExternalInput")
    wtabsh_d = nc.dram_tensor("wtabsh", [wtab_rows // N_CORES, P], bf16,
                              kind="ExternalInput")
    # all integer inputs ride in ONE u8 array (each separate H2D array pays
    # ~15-60 ms of tunnel latency): per row, bytes [0:2cp) = idx lo u16,
    # [2cp:3cp) = idx hi u8, [3cp:3cp+2c) = widx u16. idx = hi*65536 + lo
    # is rebuilt on-device (exact: 400000 < 2^24).
    pk_lo, pk_hi, pk_w = 0, 2 * cp, 3 * cp
    pk_end = pk_w + 2 * c
    packed_d = nc.dram_tensor("packed", [rows, pk_end], u8,
                              kind="ExternalInput")
    y_d = nc.dram_tensor("y", [rows, c * F], bf16, kind="ExternalOutput")
    del u16

    with TileContext(nc) as tc:
        with tc.tile_pool(name="dram", bufs=1, space="DRAM") as dram, \
             tc.tile_pool(name="gbuf", bufs=bufs) as gpool, \
             tc.tile_pool(name="wbuf", bufs=bufs) as wpool, \
             tc.tile_pool(name="ibuf", bufs=bufs) as ipool, \
             tc.tile_pool(name="ybuf", bufs=bufs) as ypool:
            # replicate pool + w_table on-device: shard -> bounce -> AllGather
            cc_in = dram.tile([shard_rows, F], bf16)
            pool_d = dram.tile([pool_rows, F], bf16, addr_space="Shared")
            nc.gpsimd.dma_start(cc_in[:], poolsh_d[:])
            nc.gpsimd.collective_compute(
                "AllGather", mybir.AluOpType.bypass,
                replica_groups=[list(range(N_CORES))],
                ins=[cc_in.opt()], outs=[pool_d.opt()],
            )
            ccw_in = dram.tile([wtab_rows // N_CORES, P], bf16)
            wtab_d = dram.tile([wtab_rows, P], bf16, addr_space="Shared")
            nc.gpsimd.dma_start(ccw_in[:], wtabsh_d[:])
            nc.gpsimd.collective_compute(
                "AllGather", mybir.AluOpType.bypass,
                replica_groups=[list(range(N_CORES))],
                ins=[ccw_in.opt()], outs=[wtab_d.opt()],
            )
            def tile_body(r0):
                rs = bass.ds(r0, 128)
                raw = ipool.tile([128, pk_end], u8, tag="raw")
                nc.sync.dma_start(out=raw[:], in_=packed_d[rs, :])
                r3 = raw[:]

                # rebuild i32 offsets from little-endian bytes:
                # it = lo_even + 256*lo_odd, then += 65536*hi (exact in fp32)
                it = ipool.tile([128, cp], i32, tag="it")
                nc.vector.scalar_tensor_tensor(
                    out=it[:], in0=r3[:, pk_lo + 1:pk_hi:2], scalar=256,
                    in1=r3[:, pk_lo:pk_hi:2],
                    op0=mybir.AluOpType.mult, op1=mybir.AluOpType.add)
                nc.vector.scalar_tensor_tensor(
                    out=it[:], in0=r3[:, pk_hi:pk_w], scalar=65536,
                    in1=it[:],
                    op0=mybir.AluOpType.mult, op1=mybir.AluOpType.add)
                wit = ipool.tile([128, c], i32, tag="wit")
                nc.vector.scalar_tensor_tensor(
                    out=wit[:], in0=r3[:, pk_w + 1:pk_end:2], scalar=256,
                    in1=r3[:, pk_w:pk_end:2],
                    op0=mybir.AluOpType.mult, op1=mybir.AluOpType.add)

                # pool gather: HW allows one descriptor per partition per
                # indirect DMA (offset AP [128,1], dest [128, F] contiguous),
                # so issue c*P instructions round-robined over 4 SWDGE queues
                g = gpool.tile([128, cp * F], bf16, tag="g")
                for s in range(cp):
                    inst = nc.gpsimd.indirect_dma_start(
                        out=g[:, s * F:(s + 1) * F], out_offset=None,
                        in_=pool_d[:],
                        in_offset=bass.IndirectOffsetOnAxis(
                            ap=it[:, s:s + 1], axis=0),
                    )
                    qi = s % 4
                    if qi:
                        inst.queue = f"qPoolDynamic{qi}"

                # w gather: c instructions of 128 descriptors x P*2 bytes
                w = wpool.tile([128, cp], bf16, tag="w")
                for s in range(c):
                    nc.gpsimd.indirect_dma_start(
                        out=w[:, s * P:(s + 1) * P], out_offset=None,
                        in_=wtab_d[:],
                        in_offset=bass.IndirectOffsetOnAxis(
                            ap=wit[:, s:s + 1], axis=0),
                    )

                # weighted multiply: g[p, sj, f] *= w[p, sj] (broadcast over f)
                g3 = g[:].rearrange("p (sj f) -> p sj f", sj=cp, f=F)
                w3 = w[:].unsqueeze(2).to_broadcast([128, cp, F])
                nc.vector.tensor_tensor(
                    out=g3, in0=g3, in1=w3, op=mybir.AluOpType.mult)

                # reduce over j (strided innermost view): [p, s, f, j] -> [p, s*f]
                # bf16 out is a final rounding only (DVE accumulates in f32);
                # harness tolerance is 2e-2, bf16 costs ~4e-3
                y_t = ypool.tile([128, c * F], bf16, tag="y")
                g4 = g[:].rearrange("p (s j f) -> p s f j", s=c, j=P, f=F)
                with nc.allow_low_precision(reason="bf16 output, 2e-2 gate"):
                    nc.vector.tensor_reduce(
                        out=y_t[:], in_=g4,
                        axis=mybir.AxisListType.X, op=mybir.AluOpType.add)

                nc.sync.dma_start(out=y_d[rs, :], in_=y_t[:])

            if USE_FOR_I:
                with tc.For_i(0, rows, 128,
                              hint_engines=(mybir.EngineType.Pool,)) as i:
                    tile_body(i)
            else:
                for t in range(t_tiles):
                    tile_body(t * 128)
    nc.finalize()
    return nc


def _prep_core_inputs(idxlo, idxhi, widx16, n0, n1, t_tiles, c):
    """Slice per-core indices, pad, reshape, and pack into one u8 array:
    per row, bytes [0:2cp) = idx lo u16, [2cp:3cp) = idx hi, [3cp:) = widx."""
    npad = t_tiles * 128 * c
    rows = t_tiles * 128
    cp = c * P
    lo_c = np.zeros((npad, P), np.uint16)
    lo_c[: n1 - n0] = idxlo[n0:n1]
    hi_c = np.zeros((npad, P), np.uint8)
    hi_c[: n1 - n0] = idxhi[n0:n1]
    w_c = np.zeros((npad,), np.uint16)
    w_c[: n1 - n0] = widx16[n0:n1]
    # neuron m = (t*128 + p)*c + s  ->  idx tile [t*128+p, s*16+j]
    packed = np.empty((rows, 3 * cp + 2 * c), np.uint8)
    packed[:, :2 * cp] = lo_c.reshape(rows, cp).view(np.uint8)
    packed[:, 2 * cp:3 * cp] = hi_c.reshape(rows, cp)
    packed[:, 3 * cp:] = w_c.reshape(rows, c).view(np.uint8)
    return packed


_NC_CACHE = {}


def _enable_jax_compile_cache():
    """Persistent XLA compilation cache so warm calls skip recompiling the
    shard_map wrapper that run_bass_via_pjrt rebuilds per call."""
    try:
        import jax

        jax.config.update("jax_compilation_cache_dir", "/tmp/jaxcache")
        jax.config.update("jax_persistent_cache_min_entry_size_bytes", -1)
        jax.config.update("jax_persistent_cache_min_compile_time_secs", 0.0)
    except Exception:
        pass


_enable_jax_compile_cache()


def kernel(values0, values1, w_table, idx, widx):
    global LAST_RESULTS
    import time as _time

    timing = bool(os.environ.get("KERNEL_TIMING"))
    tick = _time.time
    t0 = tick()
    from concourse.bass_utils import run_bass_kernel_spmd

    import ml_dtypes

    bf16 = np.dtype(ml_dtypes.bfloat16)
    pool = np.concatenate([np.asarray(values0, np.float32).astype(bf16),
                           np.asarray(values1, np.float32).astype(bf16)],
                          axis=0)
    w_table = np.asarray(w_table, np.float32).astype(bf16)
    ta = tick()
    idx32 = np.asarray(idx).astype(np.int32)
    idxlo = (idx32 & 0xFFFF).astype(np.uint16)
    idxhi = (idx32 >> 16).astype(np.uint8)
    widx16 = np.asarray(widx).astype(np.uint16)
    t1 = tick()

    if "nc" not in _NC_CACHE:
        _NC_CACHE["nc"] = build_program(T, C, 2 * M, K)
    nc = _NC_CACHE["nc"]
    t2 = tick()

    shard_rows = (2 * M) // N_CORES
    wsh_rows = K // N_CORES

    def _core_map(core):
        n0 = core * N_PER_CORE
        n1 = min(n0 + N_PER_CORE, N)
        packed = _prep_core_inputs(idxlo, idxhi, widx16, n0, n1, T, C)
        return {"poolsh": pool[core * shard_rows:(core + 1) * shard_rows],
                "wtabsh": w_table[core * wsh_rows:(core + 1) * wsh_rows],
                "packed": packed}

    from concurrent.futures import ThreadPoolExecutor

    with ThreadPoolExecutor(N_CORES) as ex:
        in_maps = list(ex.map(_core_map, range(N_CORES)))
    t3 = tick()

    kwargs = {}
    if TRACE:
        kwargs = {"trace": True, "trace_cores": [0]}
    res = run_bass_kernel_spmd(nc, in_maps, core_ids=list(range(N_CORES)),
                               **kwargs)
    LAST_RESULTS = res
    t4 = tick()

    out = np.empty((N, F), np.float32)
    for core in range(N_CORES):
        n0 = core * N_PER_CORE
        n1 = min(n0 + N_PER_CORE, N)
        y_t = res.results[core]["y"].reshape(N_PAD, F)
        out[n0:n1] = y_t[: n1 - n0]  # numpy casts bf16->f32 on assignment
    t5 = tick()
    if timing:
        print(f"[kernel timing] pool/w cast={ta-t0:.3f}s idx pack={t1-ta:.3f}s "
              f"build={t2-t1:.3f}s prep={t3-t2:.3f}s run_spmd={t4-t3:.3f}s "
              f"unshard={t5-t4:.3f}s", flush=True)
    return out


if __name__ == "__main__":
    # quick shape sanity
    print(f"T={T} tiles/core, C={C}, N_PAD={N_PAD} vs N_PER_CORE={N_PER_CORE}")
